# revision 14
# baseline (speedup 1.0000x reference)
"""Trainium2 Bass kernel for nn_MultiHeadAttention_73272142069863.

Reference semantics (softmax over the HEADS axis, dim=-1 of [b,i,j,h]):

    q = (query @ Wq).reshape(B, S, H, DH)        # biases are zero
    k = (key   @ Wk).reshape(B, S, H, DH)
    v = (value @ Wv).reshape(B, S, H, DH)
    scores = einsum("bihd,bjhd->bijh", q, k) / sqrt(DH)
    attn = softmax(scores, axis=-1)              # over h! -> Z depends on
    x = einsum("bijh,bjhd->bihd", attn, v)       #   (i,j); no flash-style
    out = x.reshape(B, S, D) @ Wo                #   deferred normalization

Sharding: core c handles batch b = c // 2 and query-row half ih = c % 2
(I = 512 rows). K/V work duplicated across the pair; no collectives.

Per-core structure (v13, 186.8us vs 250.8us baseline, same traced setup):
  - HAM warm-up burst: 24 dummy matmuls on a zeroed tile during the DMA
    ramp, so real work starts with the PE clock-gate at 2.4GHz.
  - P1: Q/K projections g-major (K split into j-halves with per-half ACT
    casts) interleaved with "diagonal" scores+exp for j-blocks 0-2.
  - P2: lag-3 jb pipeline: per iteration, softmax chain for jb-3 on DVE
    (reciprocal + normalize; Z built incrementally behind each exp),
    live attn@V for pairs 0-1 into 2 persistent PSUM banks, scores+exp
    for jb through a 3-deep PSUM rotation, V projection, incremental
    Z-tree adds interleaved behind the exp stream.
  - Tail: pairs 2-7 attn@V replay straight from SBUF, overlapped with
    the jb5-7 deferred softmax chains by re-using the freed score-
    rotation PSUM banks; then the output projection.
  - SBUF fits without HBM spill via phase-scoped LIFO pool nesting
    (pe_hi_early for diagonal j-blocks; pe_hi_late opened only after
    the Q/K staging pools close). All inputs host-packed as [128, W]
    wide-row DRAM for descriptor-efficient DMA.

Tuning history and measured dead ends: see the project memory file and
kernel_v*_*.py checkpoints alongside this file.
"""

import numpy as np
import ml_dtypes

import concourse.bass as bass
import concourse.bacc as bacc
import concourse.tile as tile
from concourse import mybir
from concourse.bass_utils import run_bass_kernel_spmd

B, S, D, H = 4, 1024, 1024, 16
DH = D // H  # 64
SCALE = 1.0 / float(np.sqrt(DH))
I = 512          # query rows per core
NCORES = 8
KC = D // 128    # 8 contraction chunks
JB = S // 128    # 8 j blocks
NPAIR = H // 2   # 8 head pairs

BF16 = mybir.dt.bfloat16
F32 = mybir.dt.float32
EXP = mybir.ActivationFunctionType.Exp


def _build():
    nc = bacc.Bacc(target_bir_lowering=False, trn_type="TRN2")

    # all inputs host-packed as [128, W] with per-partition contiguous rows
    q_d = nc.dram_tensor("qw", [128, KC * I], BF16, kind="ExternalInput")
    k_d = nc.dram_tensor("kw", [128, KC * S], BF16, kind="ExternalInput")
    v_d = nc.dram_tensor("vw", [128, KC * S], BF16, kind="ExternalInput")
    wq_d = nc.dram_tensor("wqw", [128, NPAIR * D], BF16, kind="ExternalInput")
    wk_d = nc.dram_tensor("wkw", [128, NPAIR * D], BF16, kind="ExternalInput")
    wv_d = nc.dram_tensor("wvw", [128, KC * D], BF16, kind="ExternalInput")
    wo_d = nc.dram_tensor("wow", [128, KC * D], BF16, kind="ExternalInput")
    out_d = nc.dram_tensor("out", [I, D], BF16, kind="ExternalOutput")

    with tile.TileContext(nc) as tc:
        with (
            tc.tile_pool(name="persist", bufs=KC) as pp,
            tc.tile_pool(name="pe_lo", bufs=3) as pe_lo,
            tc.tile_pool(name="pe_hi_early", bufs=3) as pe_hi_early,
            tc.tile_pool(name="pz", bufs=2) as pz,
        ):
            # ---- persistent tiles -------------------------------------
            QTs = [pp.tile([128, I], BF16, tag="QTs", name="QTs") for _ in range(KC)]
            KTs = [pp.tile([128, S], BF16, tag="KTs", name="KTs") for _ in range(KC)]
            Vs = [pp.tile([128, D], BF16, tag="Vs", name="Vs") for _ in range(JB)]
            xTs = [pp.tile([128, I], BF16, tag="xTs", name="xTs") for _ in range(NPAIR)]

            # live AV pairs 0-1 (2 PSUM banks); pairs 2-7 stored in E_hi
            E_lo = [None] * JB     # [128, 2048]  pairs 0-1
            E_hi = [None] * JB     # [128, 6144]  pairs 2-7
            rbs = [None] * JB
            zfs = [None] * JB
            ROT = {}

            def alloc_e(jb, hi_pool):
                E_lo[jb] = pe_lo.tile([128, 2048], BF16, tag="Elo", name="Elo")
                E_hi[jb] = hi_pool.tile([128, 6144], BF16, tag="Ehi", name="Ehi")

            def eslc(g, jb):
                if g < 2:
                    return E_lo[jb][:, g * 1024 : (g + 1) * 1024]
                return E_hi[jb][:, (g - 2) * 1024 : (g - 1) * 1024]

            def scores_exp(g, jb):
                jsl = slice(jb * 128, (jb + 1) * 128)
                sc = ROT["p"].tile([128, S], F32, tag="rot", name="rot")
                nc.tensor.matmul(
                    sc[:, 0:512], KTs[g][0:64, jsl], QTs[g][0:64, :],
                    start=True, stop=True, tile_position=(0, 0),
                )
                nc.tensor.matmul(
                    sc[:, 512:1024], KTs[g][64:128, jsl], QTs[g][64:128, :],
                    start=True, stop=True, tile_position=(64, 0),
                )
                nc.scalar.activation(eslc(g, jb), sc[:], EXP, scale=SCALE)

            def z_tree(jb):
                # wide-op binary tree over the 16 head chunks (few big DVE
                # ops instead of many small ones). E_lo = heads 0-3,
                # E_hi = heads 4-15 (each chunk [128, 512]).
                eh, el = E_hi[jb], E_lo[jb]
                u = pz.tile([128, 2048], BF16, tag="ztree_u", name="ztree_u",
                            bufs=1)
                # u = heads[4..7] + heads[8..11]
                nc.vector.tensor_add(u[:], eh[:, 0:2048], eh[:, 2048:4096])
                # u += heads[12..15]
                nc.vector.tensor_add(u[:], u[:], eh[:, 4096:6144])
                # u += heads[0..3]
                nc.vector.tensor_add(u[:], u[:], el[:])
                # fold in place: u[:, 0:1024] += u[:, 1024:2048]
                nc.vector.tensor_add(u[:, 0:1024], u[:, 0:1024], u[:, 1024:2048])
                zf = pz.tile([128, 512], F32, tag="zf", name="zf", bufs=3)
                nc.vector.tensor_add(zf[:], u[:, 0:512], u[:, 512:1024])
                zfs[jb] = zf

            def norm(src, jb, nblk):
                rv = rbs[jb][:].unsqueeze(1).broadcast_to([128, nblk, 512])
                nc.vector.tensor_mul(
                    src.rearrange("p (a b) -> p a b", a=nblk),
                    src.rearrange("p (a b) -> p a b", a=nblk),
                    rv,
                )

            def b_dve(jb):
                z_tree(jb)
                rf = pz.tile([128, 512], F32, tag="rf", name="rf", bufs=2)
                nc.vector.reciprocal_approx_fast(rf[:], zfs[jb][:])
                rb = pz.tile([128, 512], BF16, tag="rb", name="rb", bufs=2)
                nc.vector.tensor_copy(rb[:], rf[:])
                rbs[jb] = rb
                norm(E_lo[jb][:], jb, 4)
                norm(E_hi[jb][:], jb, 12)

            def b_av(jb, xt_ps, first, last):
                for g in range(2):
                    for p in range(2):
                        h = 2 * g + p
                        nc.tensor.matmul(
                            xt_ps[g][p * 64 : (p + 1) * 64, :],
                            Vs[jb][:, h * DH : (h + 1) * DH],
                            E_lo[jb][:, (2 * g + p) * 512 : (2 * g + p + 1) * 512],
                            start=first, stop=last,
                            tile_position=(0, p * 64),
                        )

            # ---- P1: projections g-major + diagonal scores jb0/jb1 ----
            with (
                tc.tile_pool(name="st_q", bufs=1) as pq,
                tc.tile_pool(name="st_k", bufs=1) as pk,
                tc.tile_pool(name="st_wq", bufs=3) as pwq,
                tc.tile_pool(name="st_wk", bufs=3) as pwk,
                tc.tile_pool(name="ps_p1", bufs=4, space="PSUM") as ps_p1,
            ):
                ROT["p"] = ps_p1
                qTt = pq.tile([128, KC * I], BF16, tag="qT", name="qT")
                kTt = pk.tile([128, KC * S], BF16, tag="kT", name="kT")
                wq_g = [pwq.tile([128, D], BF16, tag="wq", name="wq")
                        for _ in range(NPAIR)]
                wk_g = [pwk.tile([128, D], BF16, tag="wk", name="wk")
                        for _ in range(NPAIR)]

                # loads in consumption order, all rows contiguous
                nc.sync.dma_start(wq_g[0][:], wq_d[:, 0:D])
                nc.sync.dma_start(qTt[:], q_d[:])
                nc.sync.dma_start(wk_g[0][:], wk_d[:, 0:D])
                nc.sync.dma_start(kTt[:], k_d[:])
                for g in range(1, NPAIR):
                    nc.sync.dma_start(wq_g[g][:], wq_d[:, g * D : (g + 1) * D])
                    nc.sync.dma_start(wk_g[g][:], wk_d[:, g * D : (g + 1) * D])

                # HAM warm-up: dummy matmuls on zeroed data while inputs
                # load, so real projections start at 2.4GHz instead of 1.2
                nc.gpsimd.memset(QTs[0][:], 0.0)
                wps = ps_p1.tile([128, S], F32, tag="rot", name="rot")
                for _ in range(24):
                    nc.tensor.matmul(
                        wps[:, 0:512], QTs[0][:, 0:128], QTs[0][:],
                        start=True, stop=True,
                    )

                def q_proj(g):
                    ps = ps_p1.tile([128, S], F32, tag="rot", name="rot")
                    for kc in range(KC):
                        nc.tensor.matmul(
                            ps[:, 0:I],
                            wq_g[g][:, kc * 128 : (kc + 1) * 128],
                            qTt[:, kc * I : (kc + 1) * I],
                            start=(kc == 0), stop=(kc == KC - 1),
                        )
                    nc.vector.tensor_copy(QTs[g][:], ps[:, 0:I])

                def k_proj(g):
                    ps = ps_p1.tile([128, S], F32, tag="rot", name="rot")
                    for nh in range(2):
                        nsl = slice(nh * 512, (nh + 1) * 512)
                        for kc in range(KC):
                            nc.tensor.matmul(
                                ps[:, nsl],
                                wk_g[g][:, kc * 128 : (kc + 1) * 128],
                                kTt[:, kc * S + nh * 512 : kc * S + (nh + 1) * 512],
                                start=(kc == 0), stop=(kc == KC - 1),
                            )
                        nc.vector.tensor_copy(KTs[g][:, nsl], ps[:, nsl])

                alloc_e(0, pe_hi_early)
                alloc_e(1, pe_hi_early)
                alloc_e(2, pe_hi_early)
                for g in range(NPAIR):
                    q_proj(g)
                    k_proj(g)
                    scores_exp(g, 0)
                    scores_exp(g, 1)
                    scores_exp(g, 2)
            # q/k staging + P1 psum freed here

            with tc.tile_pool(name="pe_hi_late", bufs=5) as pe_hi_late:
                with (
                    tc.tile_pool(name="st_v", bufs=1) as pv,
                    tc.tile_pool(name="st_wv", bufs=1) as pwv,
                    tc.tile_pool(name="ps_xt", bufs=2, space="PSUM") as ps_xt,
                ):
                    vTt = pv.tile([128, KC * S], BF16, tag="vT", name="vT")
                    wvt = pwv.tile([128, KC * D], BF16, tag="wv", name="wv")
                    # issued here, but the DMA queue executes these during P1
                    nc.sync.dma_start(vTt[:], v_d[:])
                    nc.sync.dma_start(wvt[:], wv_d[:])

                    xt_ps = [ps_xt.tile([128, I], F32, tag="xt", name="xt")
                             for _ in range(2)]

                    with tc.tile_pool(name="ps_rot", bufs=3, space="PSUM") as ps_rot:
                        ROT["p"] = ps_rot

                        def v_proj(jc):
                            js = jc * 128
                            ps = ps_rot.tile([128, S], F32, tag="rot", name="rot")
                            for kc in range(KC):
                                for nh in range(2):
                                    nc.tensor.matmul(
                                        ps[:, nh * 512 : (nh + 1) * 512],
                                        vTt[:, kc * S + js : kc * S + js + 128],
                                        wvt[:, kc * D + nh * 512 : kc * D + (nh + 1) * 512],
                                        start=(kc == 0), stop=(kc == KC - 1),
                                    )
                            nc.vector.tensor_copy(Vs[jc][:], ps[:])

                        # ---- P2: steady-state jb pipeline (lag-3) ----
                        v_proj(0)
                        v_proj(1)
                        v_proj(2)
                        for jb in range(3, JB):
                            b_dve(jb - 3)
                            b_av(jb - 3, xt_ps, first=(jb == 3), last=False)
                            alloc_e(jb, pe_hi_late)
                            for g in range(NPAIR):
                                scores_exp(g, jb)
                            v_proj(jb)
                    # score/vproj psum freed: its 6 banks become the replay
                    # accumulators, so pairs 2-7 AV overlaps the jb6/jb7
                    # softmax chains
                    with tc.tile_pool(name="ps_xt2", bufs=6, space="PSUM") as ps_xt2:
                        xt_ps2 = [ps_xt2.tile([128, I], F32, tag="xt", name="xt")
                                  for _ in range(6)]

                        def av_hi(jb, first, last):
                            for g in range(2, NPAIR):
                                for p in range(2):
                                    h = 2 * g + p
                                    off = (2 * (g - 2) + p) * 512
                                    nc.tensor.matmul(
                                        xt_ps2[g - 2][p * 64 : (p + 1) * 64, :],
                                        Vs[jb][:, h * DH : (h + 1) * DH],
                                        E_hi[jb][:, off : off + 512],
                                        start=first, stop=last,
                                        tile_position=(0, p * 64),
                                    )

                        b_dve(JB - 3)
                        av_hi(0, first=True, last=False)
                        av_hi(1, first=False, last=False)
                        b_av(JB - 3, xt_ps, first=False, last=False)
                        b_dve(JB - 2)
                        av_hi(2, first=False, last=False)
                        av_hi(3, first=False, last=False)
                        b_av(JB - 2, xt_ps, first=False, last=False)
                        b_dve(JB - 1)
                        av_hi(4, first=False, last=False)
                        b_av(JB - 1, xt_ps, first=False, last=True)
                        for jb in range(5, JB):
                            av_hi(jb, first=False, last=(jb == JB - 1))
                        for g in range(2):
                            nc.vector.tensor_copy(xTs[g][:], xt_ps[g][:])
                        for g in range(2, NPAIR):
                            nc.vector.tensor_copy(xTs[g][:], xt_ps2[g - 2][:])
                # st_v/st_wv/psum freed here

                # ---- output projection (wo loaded late) ---------------
                with (
                    tc.tile_pool(name="po_rot", bufs=2, space="PSUM") as po_rot,
                    tc.tile_pool(name="pwo", bufs=1) as pwo,
                    tc.tile_pool(name="pout", bufs=2) as pout,
                ):
                    wot = pwo.tile([128, KC * D], BF16, tag="wo", name="wo")
                    nc.sync.dma_start(wot[:], wo_d[:])
                    for ic in range(I // 128):
                        isl = slice(ic * 128, (ic + 1) * 128)
                        ps = po_rot.tile([128, D], F32, tag="orot", name="orot")
                        for g in range(KC):
                            for nh in range(2):
                                nc.tensor.matmul(
                                    ps[:, nh * 512 : (nh + 1) * 512], xTs[g][:, isl],
                                    wot[:, g * D + nh * 512 : g * D + (nh + 1) * 512],
                                    start=(g == 0), stop=(g == KC - 1),
                                )
                        of = pout.tile([128, D], BF16, tag="outf", name="outf")
                        nc.vector.tensor_copy(of[:], ps[:])
                        nc.sync.dma_start(out_d[isl, :], of[:])
    nc.compile()
    return nc


_NC_CACHE = {}


def _get_nc():
    if "nc" not in _NC_CACHE:
        _NC_CACHE["nc"] = _build()
    return _NC_CACHE["nc"]


def _reference_numpy(query, key, value, mask, Wq, bq, Wk, bk, Wv, bv, Wo, bo):
    """Fallback for masked / biased inputs (reference semantics)."""
    q = (query.reshape(B * S, D) @ Wq + bq).reshape(B, S, H, DH)
    k = (key.reshape(B * S, D) @ Wk + bk).reshape(B, S, H, DH)
    v = (value.reshape(B * S, D) @ Wv + bv).reshape(B, S, H, DH)
    scores = np.einsum("bihd,bjhd->bijh", q, k).astype(np.float32) * SCALE
    scores = np.where(mask[..., None] == 0, -np.inf, scores)
    m = scores.max(axis=-1, keepdims=True)
    e = np.exp(scores - m)
    attn = e / e.sum(axis=-1, keepdims=True)
    x = np.einsum("bijh,bjhd->bihd", attn, v).reshape(B, S, D)
    return (x.reshape(B * S, D) @ Wo + bo).reshape(B, S, D).astype(np.float32)


def _row_pack(t):
    """[KC*128, W] -> [128, KC*W]: per-partition contiguous rows."""
    bf = ml_dtypes.bfloat16
    w = t.shape[1]
    out = t.reshape(KC, 128, w).transpose(1, 0, 2).reshape(128, KC * w)
    return np.ascontiguousarray(out.astype(bf))


def _pack_wqk(w):
    """[D, D] -> [128, (g c x)] so head-pair g's block is one row-slice."""
    bf = ml_dtypes.bfloat16
    t = w.reshape(KC, 128, NPAIR, 128).transpose(1, 2, 0, 3).reshape(128, NPAIR * D)
    return np.ascontiguousarray(t.astype(bf))


def kernel(query, key, value, mask, Wq, bq, Wk, bk, Wv, bv, Wo, bo):
    query = np.asarray(query, np.float32)
    key = np.asarray(key, np.float32)
    value = np.asarray(value, np.float32)
    Wq, Wk, Wv, Wo = (np.asarray(w, np.float32) for w in (Wq, Wk, Wv, Wo))
    bq, bk, bv, bo = (np.asarray(b, np.float32) for b in (bq, bk, bv, bo))
    mask_np = np.asarray(mask)

    if (not np.all(mask_np != 0)) or bq.any() or bk.any() or bv.any() or bo.any():
        return _reference_numpy(
            query, key, value, mask_np, Wq, bq, Wk, bk, Wv, bv, Wo, bo
        )

    nc = _get_nc()

    kwb = [_row_pack(key[b].T) for b in range(B)]
    vwb = [_row_pack(value[b].T) for b in range(B)]
    wqw, wkw = _pack_wqk(Wq), _pack_wqk(Wk)
    wvw, wow = _row_pack(Wv), _row_pack(Wo)

    in_maps = []
    for c in range(NCORES):
        b, ih = divmod(c, 2)
        in_maps.append({
            "qw": _row_pack(query[b, ih * I : (ih + 1) * I, :].T),
            "kw": kwb[b], "vw": vwb[b],
            "wqw": wqw, "wkw": wkw, "wvw": wvw, "wow": wow,
        })

    res = run_bass_kernel_spmd(nc, in_maps, core_ids=list(range(NCORES)))
    global LAST_RESULT
    LAST_RESULT = res
    out = np.empty((B, S, D), np.float32)
    for c in range(NCORES):
        b, ih = divmod(c, 2)
        out[b, ih * I : (ih + 1) * I, :] = np.asarray(
            res.results[c]["out"], dtype=np.float32
        )
    return out

